# revision 57
# baseline (speedup 1.0000x reference)
"""Self-contained Trainium2 kernel for nn_AgnosticResidualInteractionBlock.

kernel(**inputs) takes the FULL unsharded inputs (numpy arrays keyed as in
setup_inputs) and returns the FULL (10000, 64, 4, 2) float32 output.

Strategy (8 NeuronCores, SPMD, no collectives): edges partitioned by
receiver into per-core node ranges; per-128-receiver blocks padded to CPB
chunks of 128 edges. Each core redundantly up-projects node features to an
fp16 DRAM table (from SBUF-resident transposed features), gathers sender
rows by per-block batched indirect DMA, computes radial-MLP weights with a
stacked 128-partition block-diagonal MLP, forms per-edge products on-chip,
and performs the segment-sum as one-hot matmuls (y folded into the one-hot
matrices) accumulating in PSUM with merged 128-column matmuls. Node-level
linear/skip/gate follows with fp16 PE transposes; disjoint per-core output
slabs are concatenated on the host.
"""
import sys

sys.path.insert(0, "/opt/trn_rl_repo")

import numpy as np

N = 10000
E = 160000
MUL = 64
NATTR = 10
RDIM = 8
NAVG = 16.0
INV3 = 0.5773502691896258
NCORES = 8
NPC = N // NCORES          # 1250 nodes per core
BS = 128                   # node block size
NBLK = (NPC + BS - 1) // BS  # 10 blocks/core
P = 128

SK_NORM = 1.0 / np.sqrt(MUL * NATTR)
UP_NORM = 1.0 / np.sqrt(MUL)
LIN_NORM = (1.0 / np.sqrt(2 * MUL)) / (2.0 * NAVG)


def prep(inputs):
    """Returns (shared weight arrays dict, list of per-core array dicts, CPB)."""
    f32 = np.float32
    f16 = np.float16
    node_feats = np.asarray(inputs["node_feats"], f32)
    node_attrs = np.asarray(inputs["node_attrs"], f32)
    ear = np.asarray(inputs["edge_attrs_real"], f32)
    eai = np.asarray(inputs["edge_attrs_imag"], f32)
    ef = np.asarray(inputs["edge_feats"], f32)
    ei = np.asarray(inputs["edge_index"])
    sender = ei[0].astype(np.int64)
    receiver = ei[1].astype(np.int64)

    x0 = node_feats[:, :MUL]                       # (N,64)
    x1 = node_feats[:, MUL:].reshape(N, MUL, 3)    # (N,64,3)

    # ---- transposed node features, path-major rows: [x0 | x1_1 | x1_2 | x1_3]
    nf_T = np.empty((256, N), f32)
    nf_T[0:64] = x0.T
    for i in range(3):
        nf_T[64 * (i + 1):64 * (i + 2)] = x1[:, :, i].T

    # ---- skip path: ua-replicated rows (4 paths x 640)
    # row t*128+j within a path block corresponds to ua = t*128+j, u=ua//10, a=ua%10
    ua_u = (np.arange(640) // 10)
    ua_a = (np.arange(640) % 10)
    nf_rep = np.empty((4, 640, N), f16)
    for pth in range(4):
        nf_rep[pth] = nf_T[pth * 64 + ua_u, :].astype(f16)
    attrs_rep = node_attrs.T[ua_a, :].astype(f16)          # (640, N)

    # ---- weights (shared across cores)
    # Walrus rejects 64-row stationaries at partition base 64, so every
    # matmul uses full 128-row stationaries; the moving weights are
    # zero-padded / block-diagonal to select the intended half.
    W = {}
    W["nf_T"] = nf_T.astype(f16)                           # (256, N)
    W["nf_rep"] = nf_rep.reshape(4 * 640, N)               # (2560, N) f16
    W["attrs_rep"] = attrs_rep                             # (640, N) f16
    wu0 = (np.asarray(inputs["W_up0"], f32) * UP_NORM).astype(f16)
    wu1 = (np.asarray(inputs["W_up1"], f32) * UP_NORM).astype(f16)
    z64 = np.zeros((64, 64), f16)
    W["wupBD1"] = np.block([[wu0, z64], [z64, wu1]])        # (128,128)
    W["wupBD2"] = np.block([[wu1, z64], [z64, wu1]])        # (128,128)
    W["W2s"] = (np.asarray(inputs["W_sk_s"], f32).reshape(640, 2 * MUL) * SK_NORM).astype(f16)
    W["W2v"] = (np.asarray(inputs["W_sk_v"], f32).reshape(640, MUL) * SK_NORM).astype(f16)
    m1 = (np.asarray(inputs["M1"], f32) / np.sqrt(RDIM)).astype(f16)
    m2 = (np.asarray(inputs["M2"], f32) / 8.0).astype(f16)
    m3 = (np.asarray(inputs["M3"], f32) / 8.0).astype(f16)
    W["M1"] = m1                                            # (8,64)
    W["M2"] = m2
    W["M3"] = m3
    W["M2bd"] = np.block([[m2, z64], [z64, m2]])            # (128,128)
    W["M3bd"] = np.block([[m3, z64], [z64, m3]])            # (128,128)
    M4 = np.asarray(inputs["M4"], f32) / 8.0
    wA, wB, wC, wD = M4[:, :64], M4[:, 64:128], M4[:, 128:192], M4[:, 192:256]
    m4r = np.concatenate([wA, wC, wD * INV3, wB], axis=1).astype(f16)  # (64,256)
    z256 = np.zeros((64, 256), f16)
    W["m4lo"] = np.concatenate([m4r, z256], axis=0)         # (128,256)
    W["m4hi"] = np.concatenate([z256, m4r], axis=0)         # (128,256)
    W["m4p"] = m4r                                          # (64,256)
    wls = (np.asarray(inputs["W_lin_s"], f32) * LIN_NORM).astype(f16)  # (128,128)
    z128 = np.zeros((64, 128), f16)
    W["wsA"] = np.concatenate([wls[0:64], z128], axis=0)    # (128,128)
    W["wsB"] = np.concatenate([wls[64:128], z128], axis=0)  # (128,128)
    wlv = (np.asarray(inputs["W_lin_v"], f32) * LIN_NORM).astype(f16)  # (128,64)
    # reference mv rows: [0:64]=B-path (xs0*wB x y1), [64:128]=C-path
    # (xs1*y0*wC).  vC transposed blocks pair with Wlv[64:128], vB blocks
    # (always rows 64:128 of their group) pair with Wlv[0:64].
    W["wvch"] = np.concatenate([z64, wlv[64:128]], axis=0)  # (128,64)
    W["wvcl"] = np.concatenate([wlv[64:128], z64], axis=0)  # (128,64)
    W["wvb"] = np.concatenate([z64, wlv[0:64]], axis=0)     # (128,64)
    W["iota"] = np.broadcast_to(np.arange(P, dtype=f16), (P, P)).copy()
    W["ident16"] = np.eye(P, dtype=f16)

    # ---- edge bucketing (64-node receiver sub-blocks)
    core_of = receiver // NPC
    rlocal = receiver - core_of * NPC
    blk_of = rlocal // 64
    rblk = rlocal - blk_of * 64

    # y8 per edge: [y0r, y1r_1..3, y0i, y1i_1..3]
    y8 = np.concatenate([ear[:, :1], ear[:, 1:], eai[:, :1], eai[:, 1:]], axis=1).astype(f32)

    buckets = [[[] for _ in range(2 * NBLK)] for _ in range(NCORES)]
    order = np.lexsort((blk_of, core_of))
    for e in order:
        buckets[core_of[e]][blk_of[e]].append(e)

    CPB = 0
    for k in range(NCORES):
        for b in range(2 * NBLK):
            CPB = max(CPB, (len(buckets[k][b]) + P - 1) // P)

    EPB = CPB * P              # edge slots per 64-node sub-block
    NB2 = 2 * NBLK
    cores = []
    for k in range(NCORES):
        send_sw = np.zeros((NB2, P, CPB), np.int32)
        rloc_sw = np.zeros((NB2, P, CPB), f32)
        y8_sw = np.zeros((NB2, P, CPB * 8), f32)       # col c*8+j
        ef_sw = np.zeros((NB2, 8, EPB), f16)           # col c*128+p
        for b in range(NB2):
            es = buckets[k][b]
            ne = len(es)
            if ne == 0:
                continue
            es = np.array(es, np.int64)
            slot = np.arange(ne)
            c, p = slot // P, slot % P
            send_sw[b, p, c] = sender[es]
            rloc_sw[b, p, c] = rblk[es]
            for j in range(8):
                y8_sw[b, p, c * 8 + j] = y8[es, j]
            ef_sw[b, :, c * P + p] = ef[es].astype(f16)
        cores.append(dict(
            send_sw=send_sw, rloc_sw=rloc_sw, y8_sw=y8_sw, ef_sw=ef_sw,
        ))
    return W, cores, CPB


def make_inmaps(W, cores, CPB):
    """Per-core input dicts matching bass_kernel.build() tensor names."""
    f16 = np.float16
    NPCpad = NBLK * P
    shared = {
        "nf_T": W["nf_T"], "wupBD1": W["wupBD1"], "wupBD2": W["wupBD2"],
        "W2s": W["W2s"], "W2v": W["W2v"], "M1": W["M1"], "M2": W["M2"],
        "M3": W["M3"], "M2bd": W["M2bd"], "M3bd": W["M3bd"],
        "m4lo": W["m4lo"], "m4hi": W["m4hi"], "m4p": W["m4p"],
        "wsA": W["wsA"], "wsB": W["wsB"],
        "wvch": W["wvch"], "wvcl": W["wvcl"], "wvb": W["wvb"],
        "iota": W["iota"], "ident16": W["ident16"],
    }
    in_maps = []
    for k in range(NCORES):
        ca = cores[k]
        lo = k * NPC
        nf_rep_k = np.zeros((4 * 640, NPCpad), f16)
        nf_rep_k[:, :NPC] = W["nf_rep"][:, lo:lo + NPC]
        attrs_rep_k = np.zeros((640, NPCpad), f16)
        attrs_rep_k[:, :NPC] = W["attrs_rep"][:, lo:lo + NPC]
        m = dict(shared)
        m["nf_rep"] = nf_rep_k
        m["attrs_rep"] = attrs_rep_k
        m["send_sw"] = ca["send_sw"].reshape(2 * NBLK * P, CPB)
        m["rloc_sw"] = ca["rloc_sw"].reshape(2 * NBLK * P, CPB)
        m["y8_sw"] = ca["y8_sw"].reshape(2 * NBLK * P, CPB * 8)
        m["ef_sw"] = ca["ef_sw"].reshape(2 * NBLK * 8, CPB * P)
        in_maps.append(m)
    return in_maps


import concourse.bass as bass
import concourse.bacc as bacc
import concourse.tile as tile
from concourse import mybir

F32 = mybir.dt.float32
F16 = mybir.dt.float16
I32 = mybir.dt.int32
AF = mybir.ActivationFunctionType
ALU = mybir.AluOpType

NPCpad = NBLK * P
NCHUNK_UP = (N + P - 1) // P      # 79 global chunks for up-projection


def ap_view(ap, offset_elems, dims):
    """Build a free-dim view of a tile AP: keeps partition dim, replaces the
    free dims with `dims` ([step, num] pairs, steps in elements) at an
    element offset from the tile base."""
    return bass.AP(ap.tensor, ap.offset + offset_elems, [ap.ap[0]] + list(dims))


import os
GATHER_BLOCK = os.environ.get("K_GATHER_BLOCK", "1") == "1"
F16T = os.environ.get("K_F16T", "1") == "1"
MLPSTACK = os.environ.get("K_MLPSTACK", "1") == "1"
SY_8TS = os.environ.get("K_SY8TS", "0") == "1"
SY_MODE = int(os.environ.get("K_SYMODE", "0"))


def build(CPB):
    nc = bacc.Bacc("TRN2", target_bir_lowering=False, debug=False,
                   num_devices=NCORES)
    EPB = CPB * P
    NFULL = EPB // 1024                # stacked MLP subs of 1024 edges
    REM = EPB - NFULL * 1024           # remainder edges (plain 64-row path)

    def din(name, shape, dt):
        return nc.dram_tensor(name, list(shape), dt, kind="ExternalInput").ap()

    # ---------------- I/O ----------------
    nf_T = din("nf_T", (256, N), F16)
    nf_rep = din("nf_rep", (4 * 640, NPCpad), F16)        # per-core slice
    attrs_rep = din("attrs_rep", (640, NPCpad), F16)      # per-core slice
    wupBD1_in = din("wupBD1", (128, 128), F16)
    wupBD2_in = din("wupBD2", (128, 128), F16)
    W2s = din("W2s", (640, 128), F16)
    W2v = din("W2v", (640, 64), F16)
    M1 = din("M1", (8, 64), F16)
    M2 = din("M2", (64, 64), F16)
    M3 = din("M3", (64, 64), F16)
    M2bd = din("M2bd", (128, 128), F16)
    M3bd = din("M3bd", (128, 128), F16)
    m4lo_in = din("m4lo", (128, 256), F16)
    m4hi_in = din("m4hi", (128, 256), F16)
    m4p_in = din("m4p", (64, 256), F16)
    wsA_in = din("wsA", (128, 128), F16)
    wsB_in = din("wsB", (128, 128), F16)
    wvch_in = din("wvch", (128, 64), F16)
    wvcl_in = din("wvcl", (128, 64), F16)
    wvb_in = din("wvb", (128, 64), F16)
    iota_in = din("iota", (P, P), F16)
    ident_in = din("ident16", (P, P), F16)
    send_sw = din("send_sw", (2 * NBLK * P, CPB), I32)
    rloc_sw = din("rloc_sw", (2 * NBLK * P, CPB), F32)
    y8_sw = din("y8_sw", (2 * NBLK * P, CPB * 8), F32)
    ef_sw = din("ef_sw", (2 * NBLK * 8, EPB), F16)

    out = nc.dram_tensor("out", [NBLK * P, 512], F32, kind="ExternalOutput").ap()
    # x_up scratch (node-major, fp16); dedicated tensor => offset-0 AP for
    # the indirect gather.
    x_up = nc.dram_tensor("x_up", [N, 256], F16).ap()

    with tile.TileContext(nc) as tc:
        with tc.tile_pool(name="const", bufs=1) as cpool:
            iota_t = cpool.tile([P, P], F16)
            nc.sync.dma_start(iota_t[:], iota_in[:])
            ident_t = cpool.tile([P, P], F16)
            nc.sync.dma_start(ident_t[:], ident_in[:])
            ident32_t = None
            if not F16T:
                from concourse.masks import make_identity
                ident32_t = cpool.tile([P, P], F32)
                make_identity(nc, ident32_t[:])
            wupBD1 = cpool.tile([128, 128], F16)
            nc.sync.dma_start(wupBD1[:], wupBD1_in[:])
            wupBD2 = cpool.tile([128, 128], F16)
            nc.sync.dma_start(wupBD2[:], wupBD2_in[:])
            w2s_t = cpool.tile([P, 5, 128], F16)
            nc.sync.dma_start(w2s_t[:], W2s.rearrange("(t p) v -> p t v", p=P))
            w2v_t = cpool.tile([P, 5, 64], F16)
            nc.sync.dma_start(w2v_t[:], W2v.rearrange("(t p) v -> p t v", p=P))
            m1_t = cpool.tile([8, 64], F16)
            nc.sync.dma_start(m1_t[:], M1[:])
            m2_t = cpool.tile([64, 64], F16)
            nc.sync.dma_start(m2_t[:], M2[:])
            m3_t = cpool.tile([64, 64], F16)
            nc.sync.dma_start(m3_t[:], M3[:])
            m2bd_t = cpool.tile([128, 128], F16)
            nc.sync.dma_start(m2bd_t[:], M2bd[:])
            m3bd_t = cpool.tile([128, 128], F16)
            nc.sync.dma_start(m3bd_t[:], M3bd[:])
            m4lo_t = cpool.tile([128, 256], F16)
            nc.sync.dma_start(m4lo_t[:], m4lo_in[:])
            m4hi_t = cpool.tile([128, 256], F16)
            nc.sync.dma_start(m4hi_t[:], m4hi_in[:])
            m4p_t = cpool.tile([64, 256], F16)
            nc.sync.dma_start(m4p_t[:], m4p_in[:])
            wsA = cpool.tile([128, 128], F16)
            nc.sync.dma_start(wsA[:], wsA_in[:])
            wsB = cpool.tile([128, 128], F16)
            nc.sync.dma_start(wsB[:], wsB_in[:])
            wvch = cpool.tile([128, 64], F16)
            nc.sync.dma_start(wvch[:], wvch_in[:])
            wvcl = cpool.tile([128, 64], F16)
            nc.sync.dma_start(wvcl[:], wvcl_in[:])
            wvb = cpool.tile([128, 64], F16)
            nc.sync.dma_start(wvb[:], wvb_in[:])
            sc_s_t = cpool.tile([P, NBLK, 128], F32)
            sc_v_t = cpool.tile([P, NBLK, 192], F32)
            # SBUF-resident transposed node features (2 path-pair tiles)
            nfs0 = cpool.tile([128, N], F16)
            nc.scalar.dma_start(nfs0[:], nf_T[0:128, :])
            nfs1 = cpool.tile([128, N], F16)
            nc.scalar.dma_start(nfs1[:], nf_T[128:256, :])

            # ------- flat pools: A1/A2/B share psum tags so phases overlap ----
            import contextlib as _ctx
            _es = _ctx.ExitStack()
            pA = _es.enter_context(tc.tile_pool(name="pA", bufs=16))
            pS = _es.enter_context(tc.tile_pool(name="pS", bufs=3))
            pE = _es.enter_context(tc.tile_pool(name="pE", bufs=3))
            pEd = _es.enter_context(tc.tile_pool(name="pEd", bufs=4))
            pH = _es.enter_context(tc.tile_pool(name="pH", bufs=2))
            pC = _es.enter_context(tc.tile_pool(name="pC", bufs=2))
            msgp = _es.enter_context(tc.tile_pool(name="msgp", bufs=1, space="PSUM"))
            twhp = _es.enter_context(tc.tile_pool(name="twh", bufs=4, space="PSUM"))
            cpp = _es.enter_context(tc.tile_pool(name="cp", bufs=1, space="PSUM"))
            with _es:
                # ------- phase A1: redundant up-projection of all N nodes ----
                for g in range(NCHUNK_UP):
                    c0 = g * P
                    nn = min(P, N - c0)
                    ps = twhp.tile([nn, 256], F32, tag="twh")
                    nc.tensor.matmul(ps[:, 0:128], nfs0[:, c0:c0 + nn],
                                     wupBD1[:], start=True, stop=True)
                    nc.tensor.matmul(ps[:, 128:256], nfs1[:, c0:c0 + nn],
                                     wupBD2[:], start=True, stop=True)
                    xs16 = pA.tile([nn, 256], F16, tag="xup16")
                    if g % 2 == 0:
                        nc.scalar.activation(xs16[:], ps[:], AF.Copy)
                    else:
                        nc.vector.tensor_copy(out=xs16[:], in_=ps[:])
                    nc.sync.dma_start(x_up[c0:c0 + nn, :], xs16[:])

                # ------- phase A2: skip path (own nodes) ----
                nf_rep_r = nf_rep.rearrange("(q t p) n -> p q t n", q=4, p=P)
                attrs_rep_r = attrs_rep.rearrange("(t p) n -> p t n", p=P)
                for j in range(NBLK):
                    c0 = j * P
                    at = pS.tile([P, 5, 128], F16, tag="at")
                    nc.sync.dma_start(at[:], attrs_rep_r[:, :, c0:c0 + P])
                    zt = []
                    for pth in range(4):
                        xr = pS.tile([P, 5, 128], F16, tag=f"xr{pth}")
                        nc.sync.dma_start(
                            xr[:], nf_rep_r[:, pth:pth + 1, :, c0:c0 + P])
                        z = pS.tile([P, 5, 128], F16, tag=f"z{pth}")
                        nc.vector.tensor_tensor(out=z[:], in0=xr[:], in1=at[:],
                                                op=ALU.mult)
                        zt.append(z)
                    ps_sv = twhp.tile([P, 320], F32, tag="twh")
                    for t in range(5):
                        nc.tensor.matmul(ps_sv[:, 0:128], zt[0][:, t, :],
                                         w2s_t[:, t, :],
                                         start=(t == 0), stop=(t == 4))
                    for i in range(3):
                        for t in range(5):
                            nc.tensor.matmul(
                                ps_sv[:, 128 + i * 64:128 + (i + 1) * 64],
                                zt[1 + i][:, t, :], w2v_t[:, t, :],
                                start=(t == 0), stop=(t == 4))
                    nc.scalar.activation(sc_s_t[:, j, :], ps_sv[:, 0:128], AF.Copy)
                    nc.scalar.activation(sc_v_t[:, j, :], ps_sv[:, 128:320], AF.Copy)

                # ------- phases B + C per block -------
                # each 128-node C-block = two 64-node receiver sub-blocks;
                # the scatter writes sub-block h into msg partitions
                # [64h:64h+64] (half-width one-hots => half the DVE work)
                for b in range(NBLK):
                    msg_r = msgp.tile([P, 512], F32, tag="msgr")
                    msg_i = msgp.tile([P, 512], F32, tag="msgi")
                    msg_b = msgp.tile([P, 256], F32, tag="msgb")
                    for half in (0, 1):
                        b2 = 2 * b + half
                        hp0 = 64 * half
                        idx_t = pE.tile([P, CPB], I32, tag="idx")
                        nc.sync.dma_start(idx_t[:], send_sw[b2 * P:(b2 + 1) * P, :])
                        rloc_t = pE.tile([P, CPB], F32, tag="rloc")
                        nc.sync.dma_start(rloc_t[:], rloc_sw[b2 * P:(b2 + 1) * P, :])
                        y8_t = pE.tile([P, CPB * 8], F32, tag="y8")
                        nc.sync.dma_start(y8_t[:], y8_sw[b2 * P:(b2 + 1) * P, :])
                        ef_t = pE.tile([8, EPB], F16, tag="ef")
                        nc.sync.dma_start(ef_t[:], ef_sw[b2 * 8:(b2 + 1) * 8, :])
                        # sender-row gathers (one per chunk)
                        xs_blk = pE.tile([P, CPB, 256], F16, tag="xs")
                        for c in range(CPB):
                            nc.gpsimd.indirect_dma_start(
                                out=xs_blk[:, c, :], out_offset=None, in_=x_up,
                                in_offset=bass.IndirectOffsetOnAxis(
                                    ap=idx_t[:, c:c + 1], axis=0))

                        # ---- radial MLP ----
                        nfull = NFULL if MLPSTACK else 0
                        rem = EPB - nfull * 1024
                        h3s = pH.tile([128, max(nfull, 1) * 512], F16, tag="h3s")
                        for ss in range(nfull):
                            e0 = ss * 1024
                            h1ps = twhp.tile([128, 512], F32, tag="twh")
                            nc.tensor.matmul(h1ps[0:64, :], m1_t[:],
                                             ef_t[:, e0:e0 + 512],
                                             start=True, stop=True,
                                             tile_position=(0, 0))
                            nc.tensor.matmul(h1ps[64:128, :], m1_t[:],
                                             ef_t[:, e0 + 512:e0 + 1024],
                                             start=True, stop=True,
                                             tile_position=(0, 64))
                            h1 = pH.tile([128, 512], F16, tag="h1")
                            nc.scalar.activation(h1[:], h1ps[:], AF.Silu)
                            h2ps = twhp.tile([128, 512], F32, tag="twh")
                            nc.tensor.matmul(h2ps[:], m2bd_t[:], h1[:],
                                             start=True, stop=True)
                            h2 = pH.tile([128, 512], F16, tag="h1")
                            nc.scalar.activation(h2[:], h2ps[:], AF.Silu)
                            h3ps = twhp.tile([128, 512], F32, tag="twh")
                            nc.tensor.matmul(h3ps[:], m3bd_t[:], h2[:],
                                             start=True, stop=True)
                            nc.scalar.activation(h3s[:, e0 // 2:e0 // 2 + 512],
                                                 h3ps[:], AF.Silu)
                        # plain 64-row remainder subs (<=512 edges each)
                        h3r = None
                        if rem > 0:
                            h3r = pH.tile([64, rem], F16, tag="h3r")
                            r0 = nfull * 1024
                            off = 0
                            while off < rem:
                                ee = min(512, rem - off)
                                e0 = r0 + off
                                hps = twhp.tile([64, 512], F32, tag="twh")
                                nc.tensor.matmul(hps[:, :ee], m1_t[:],
                                                 ef_t[:, e0:e0 + ee],
                                                 start=True, stop=True)
                                h1r = pH.tile([64, 512], F16, tag="h1")
                                nc.scalar.activation(h1r[:, :ee], hps[:, :ee], AF.Silu)
                                hps2 = twhp.tile([64, 512], F32, tag="twh")
                                nc.tensor.matmul(hps2[:, :ee], m2_t[:], h1r[:, :ee],
                                                 start=True, stop=True)
                                h2r = pH.tile([64, 512], F16, tag="h1")
                                nc.scalar.activation(h2r[:, :ee], hps2[:, :ee], AF.Silu)
                                hps3 = twhp.tile([64, 512], F32, tag="twh")
                                nc.tensor.matmul(hps3[:, :ee], m3_t[:], h2r[:, :ee],
                                                 start=True, stop=True)
                                nc.scalar.activation(h3r[:, off:off + ee],
                                                     hps3[:, :ee], AF.Silu)
                                off += ee

                        for c in range(CPB):
                            # per-chunk TP weights tw = h3^T @ M4'
                            e0 = c * 128
                            if e0 < nfull * 1024:
                                sub, w = e0 // 1024, e0 % 1024
                                col = sub * 512 + (w % 512)
                                h3sl = h3s[:, col:col + 128]
                                m4sel = m4lo_t if w < 512 else m4hi_t
                            else:
                                off = e0 - nfull * 1024
                                h3sl = h3r[:, off:off + 128]
                                m4sel = m4p_t
                            twps = twhp.tile([P, 256], F32, tag="twh")
                            nc.tensor.matmul(twps[:], h3sl, m4sel[:],
                                             start=True, stop=True)
                            tws = pEd.tile([P, 256], F16, tag="tws")
                            nc.scalar.activation(tws[:], twps[:], AF.Copy)

                            # half-width one-hot scatter matrices (y folded in)
                            sy8 = pEd.tile([P, 8, 64], F16, tag="sy8")
                            s1h = pEd.tile([P, 64], F16, tag="s1h")
                            nc.vector.tensor_scalar(
                                out=s1h[:], in0=iota_t[:, 0:64],
                                scalar1=rloc_t[:, c:c + 1], scalar2=None,
                                op0=ALU.is_equal)
                            nc.scalar.activation(
                                sy8[:, 0, :], s1h[:], AF.Copy,
                                scale=y8_t[:, c * 8:c * 8 + 1])
                            nc.scalar.activation(
                                sy8[:, 4, :], s1h[:], AF.Copy,
                                scale=y8_t[:, c * 8 + 4:c * 8 + 5])
                            nc.vector.tensor_tensor(
                                out=sy8[:, 1:4, :],
                                in0=ap_view(s1h[:], 0, [[0, 3], [1, 64]]),
                                in1=ap_view(y8_t[:], c * 8 + 1, [[1, 3], [0, 64]]),
                                op=ALU.mult)
                            nc.vector.tensor_tensor(
                                out=sy8[:, 5:8, :],
                                in0=ap_view(s1h[:], 0, [[0, 3], [1, 64]]),
                                in1=ap_view(y8_t[:], c * 8 + 5, [[1, 3], [0, 64]]),
                                op=ALU.mult)

                            # per-edge products, layout:
                            # [s0 64 | vC 192 | (D0|B) 128 | (D1|B) 128 | (D2|B) 128]
                            xs = xs_blk[:, c, :]
                            prod = pEd.tile([P, 640], F16, tag="prod")
                            nc.vector.tensor_tensor(
                                out=prod[:, 0:64], in0=xs[:, 0:64],
                                in1=tws[:, 0:64], op=ALU.mult)
                            nc.vector.tensor_tensor(
                                out=prod[:, 64:256],
                                in0=ap_view(xs, 64, [[64, 3], [1, 64]]),
                                in1=ap_view(tws[:], 64, [[0, 3], [1, 64]]),
                                op=ALU.mult)
                            nc.vector.tensor_tensor(
                                out=ap_view(prod[:], 256, [[128, 3], [1, 64]]),
                                in0=ap_view(xs, 64, [[64, 3], [1, 64]]),
                                in1=ap_view(tws[:], 128, [[0, 3], [1, 64]]),
                                op=ALU.mult)
                            nc.vector.tensor_tensor(
                                out=ap_view(prod[:], 320, [[128, 3], [1, 64]]),
                                in0=ap_view(xs, 0, [[0, 3], [1, 64]]),
                                in1=ap_view(tws[:], 192, [[0, 3], [1, 64]]),
                                op=ALU.mult)

                            st0 = (c == 0)
                            st1 = (c == CPB - 1)
                            for cv, msg in ((0, msg_r), (1, msg_i)):
                                k0 = cv * 4
                                nc.tensor.matmul(msg[hp0:hp0 + 64, 0:256],
                                                 sy8[:, k0, :],
                                                 prod[:, 0:256], start=st0, stop=st1,
                                                 skip_group_check=True,
                                                 tile_position=(0, hp0))
                                nc.tensor.matmul(msg[hp0:hp0 + 64, 256:384],
                                                 sy8[:, k0 + 1, :],
                                                 prod[:, 256:384], start=st0, stop=st1,
                                                 skip_group_check=True,
                                                 tile_position=(0, hp0))
                                nc.tensor.matmul(msg[hp0:hp0 + 64, 384:512],
                                                 sy8[:, k0 + 2, :],
                                                 prod[:, 384:512], start=st0, stop=st1,
                                                 skip_group_check=True,
                                                 tile_position=(0, hp0))
                                nc.tensor.matmul(
                                    msg_b[hp0:hp0 + 64, cv * 128:cv * 128 + 128],
                                    sy8[:, k0 + 3, :],
                                    prod[:, 512:640], start=st0, stop=st1,
                                    skip_group_check=True,
                                    tile_position=(0, hp0))

                    # ---- phase C ----
                    out_t = pC.tile([P, 512], F32, tag="out")
                    for cv, msg in ((0, msg_r), (1, msg_i)):
                        mtT = pC.tile([P, 5, P], F16, tag="mtT")
                        if F16T:
                            msb = pC.tile([P, 640], F16, tag="msb")
                            nc.scalar.activation(msb[:, 0:512], msg[:], AF.Copy)
                            nc.scalar.activation(msb[:, 512:640],
                                                 msg_b[:, cv * 128:cv * 128 + 128],
                                                 AF.Copy)
                            # 5 transposes of 128-col groups (f16)
                            tp = cpp.tile([P, 5, P], F16, tag="cp1")
                            for t in range(5):
                                nc.tensor.transpose(tp[:, t, :],
                                                    msb[:, 128 * t:128 * (t + 1)],
                                                    ident_t[:])
                            nc.scalar.activation(mtT[:], tp[:], AF.Copy)
                        else:
                            msb = pC.tile([P, 640], F32, tag="msb")
                            nc.vector.tensor_copy(out=msb[:, 0:512], in_=msg[:])
                            nc.vector.tensor_copy(
                                out=msb[:, 512:640],
                                in_=msg_b[:, cv * 128:cv * 128 + 128])
                            for t in range(5):
                                tp32 = cpp.tile([P, P], F32, tag="tp")
                                nc.tensor.transpose(tp32[:],
                                                    msb[:, 128 * t:128 * (t + 1)],
                                                    ident32_t[:])
                                nc.scalar.activation(mtT[:, t, :], tp32[:], AF.Copy)
                        # linear: groups g0=[s0;vC0] g1=[vC1;vC2] g2=[sD0;vB0]
                        #         g3=[sD1;vB1] g4=[sD2;vB2]
                        lslv = cpp.tile([P, 320], F32, tag="cp1")
                        ls = lslv[:, 0:128]
                        nc.tensor.matmul(ls, mtT[:, 0, :], wsA[:],
                                         start=True, stop=False)
                        nc.tensor.matmul(ls, mtT[:, 2, :], wsB[:],
                                         start=False, stop=False)
                        nc.tensor.matmul(ls, mtT[:, 3, :], wsB[:],
                                         start=False, stop=False)
                        nc.tensor.matmul(ls, mtT[:, 4, :], wsB[:],
                                         start=False, stop=True)
                        lv0 = lslv[:, 128:192]
                        nc.tensor.matmul(lv0, mtT[:, 0, :], wvch[:],
                                         start=True, stop=False)
                        nc.tensor.matmul(lv0, mtT[:, 2, :], wvb[:],
                                         start=False, stop=True)
                        lv1 = lslv[:, 192:256]
                        nc.tensor.matmul(lv1, mtT[:, 1, :], wvcl[:],
                                         start=True, stop=False)
                        nc.tensor.matmul(lv1, mtT[:, 3, :], wvb[:],
                                         start=False, stop=True)
                        lv2 = lslv[:, 256:320]
                        nc.tensor.matmul(lv2, mtT[:, 1, :], wvch[:],
                                         start=True, stop=False)
                        nc.tensor.matmul(lv2, mtT[:, 4, :], wvb[:],
                                         start=False, stop=True)

                        if cv == 0:
                            stot = pC.tile([P, 128], F32, tag="stot")
                            vtot = pC.tile([P, 192], F32, tag="vtot")
                            nc.vector.tensor_tensor(out=stot[:], in0=lslv[:, 0:128],
                                                    in1=sc_s_t[:, b, :], op=ALU.add)
                            nc.vector.tensor_tensor(out=vtot[:], in0=lslv[:, 128:320],
                                                    in1=sc_v_t[:, b, :], op=ALU.add)
                            s_ap, v_ap = stot[:], vtot[:]
                        else:
                            s_ap, v_ap = lslv[:, 0:128], lslv[:, 128:320]
                        g = pC.tile([P, 64], F32, tag="g")
                        nc.scalar.activation(g[:], ap_view(s_ap, 64, [[1, 64]]),
                                             AF.Silu)
                        # scal -> out cols cv+2f, f in [0,64)
                        oap = out_t[:]
                        scal_out = bass.AP(oap.tensor, oap.offset + cv,
                                           [oap.ap[0], [2, 64]])
                        nc.scalar.activation(scal_out,
                                             ap_view(s_ap, 0, [[1, 64]]), AF.Silu)
                        # gated v: out col 2*(64+3u+i)+cv <- vtot[i*64+u]*g[u]
                        gout = bass.AP(oap.tensor, oap.offset + 128 + cv,
                                       [oap.ap[0], [6, 64], [2, 3]])
                        vsrc = ap_view(v_ap, 0, [[1, 64], [64, 3]])
                        gsrc = ap_view(g[:], 0, [[1, 64], [0, 3]])
                        nc.vector.tensor_tensor(out=gout, in0=vsrc, in1=gsrc,
                                                op=ALU.mult)
                    nc.sync.dma_start(out[b * P:(b + 1) * P, :], out_t[:])
    nc.finalize()
    return nc


# ---------------------------------------------------------------------------
# entry point
# ---------------------------------------------------------------------------
_CACHE = {}


def kernel(**inputs):
    from concourse.bass_utils import run_bass_kernel_spmd

    W, cores, CPB = prep(inputs)
    in_maps = make_inmaps(W, cores, CPB)
    if CPB not in _CACHE:
        _CACHE[CPB] = build(CPB)
    nc = _CACHE[CPB]
    res = run_bass_kernel_spmd(nc, in_maps, core_ids=list(range(NCORES)))
    outs = [res.results[k]["out"][:NPC] for k in range(NCORES)]
    full = np.concatenate(outs, axis=0).astype(np.float32)
    return full.reshape(N, MUL, 4, 2)
